# revision 9
# baseline (speedup 1.0000x reference)
"""Trainium2 Bass kernel for nn_ClothGraphConvNetwork_MLPDecoder.

8 NeuronCores, data-parallel over batch (2 batches/core), no collectives.

v2 design (vs v1 baseline ~950us/iter):
- Host precomputes everything that depends only on (weights, image): the
  per-batch lin0 image projection v_b, the analytic b0 GroupNorm1
  coefficients (vb folded into the bias), the rank-4 skip projection
  skv = skW @ W0v^T and its per-batch bias.  The 2048-wide lin0/skip
  matmuls never run on device.
- All weights SBUF-resident in bf16, loaded once outside the repeat
  loop in a handful of large DMAs.
- Activations bf16 (PSUM accumulation f32).  GroupNorm statistics come
  from accum_out sums fused into the PSUM evacuation instructions plus
  one Square pass per row; bn_stats is gone.
- Residual adds fused into the lin2 evacuation (DVE scalar_tensor_tensor
  psum+bias+x_old with accum), replacing the identity matmuls.
- Elementwise work split between Activation and DVE engines by a greedy
  load balancer; Pool only issues DMAs.
- The two local batches are emitted interleaved phase-by-phase so each
  batch's GroupNorm chain latency hides under the other batch's matmuls.
- Graph aggregation: dense adjacency matmul.  AGG_FP8 uses fp8e4m3 with
  DoubleRow perf mode and a two-pass error-feedback split
  (sup*s = q1 + r, both fp8) so the quantization error is ~fp8^2;
  otherwise plain f16.
"""

import contextlib

import numpy as np
import ml_dtypes

import concourse.bass as bass
import concourse.tile as tile
from concourse import bacc, mybir
from concourse.bass_utils import run_bass_kernel_spmd

F32R = mybir.dt.float32r
F32 = mybir.dt.float32
F16 = mybir.dt.float16
FP8 = mybir.dt.float8e4
AF = mybir.ActivationFunctionType
ALU = mybir.AluOpType

B, N, DEG = 16, 1723, 8
C, L, H = 512, 5, 256
NP = 1724
NCORES = 8
BLOC = B // NCORES
NT = 14                 # 128-vertex tiles
NT2 = 7                 # 256-vertex double tiles (fp8 DoubleRow)
MCH = [(0, 432), (432, 432), (864, 432), (1296, 428)]   # matmul chunks
ROWM = 1296             # full-row main piece; tail is [1296:1723]
SUPS = 64.0             # fp8 sup scale
AGG_FP8 = True

# wcat (bf16 weights) column layout
_WC = {}
_pos = 0
def _wslot(name, w):
    global _pos
    _WC[name] = (_pos, w)
    _pos += w
for _kt in range(8):
    _wslot(f"b0l1_{_kt}", H)
_wslot("b0cw_0", H); _wslot("b0cw_1", H)
_wslot("b0l2_0", C); _wslot("b0l2_1", C)
for _i in range(L):
    for _ct in range(4):
        _wslot(f"bl1_{_i}_{_ct}", H)
    _wslot(f"bcw_{_i}_0", H); _wslot(f"bcw_{_i}_1", H)
    _wslot(f"bl2_{_i}_0", C); _wslot(f"bl2_{_i}_1", C)
for _kt in range(4):
    _wslot(f"h1_{_kt}", 64)
_wslot("h2", 32)
_wslot("h3", 4)
WCOLS = _pos


def _param_layout():
    items = [("b0_lin1_b", 256), ("b0_n1_g", 256), ("b0_n1_b", 256),
             ("b0_conv_b", 256), ("b0_n2_g", 256), ("b0_n2_b", 256),
             ("b0_lin2_b", 512),
             ("blk0_pre_g", 512), ("blk0_pre_b", 512)]
    for i in range(L):
        items += [(f"lin1_b{i}", 256), (f"n1_g{i}", 256), (f"n1_b{i}", 256),
                  (f"conv_b{i}", 256), (f"n2_g{i}", 256), (f"n2_b{i}", 256),
                  (f"lin2_b{i}", 512)]
        if i < L - 1:
            items += [(f"pre_g{i + 1}", 512), (f"pre_b{i + 1}", 512)]
    items += [("h1_b", 64), ("h2_b", 32), ("hn_g", 32), ("hn_b", 32),
              ("h3_b", 3)]
    for b in range(BLOC):
        items += [(f"ab0a{b}", 1024), (f"ab0b{b}", 1024), (f"skb{b}", 512)]
    idx = {}
    pos = 0
    for name, ln in items:
        for t in range((ln + 127) // 128):
            idx[(name, t)] = pos
            pos += 1
    return items, idx, pos


PARAM_ITEMS, PIDX, NSLOT = _param_layout()
PHASES = []


class _Bal:
    """Greedy Act/DVE load balancer (ns units)."""

    def __init__(self):
        self.a = 0.0
        self.v = 0.0

    def pick(self, ca, cv):
        if self.a + ca <= self.v + cv:
            self.a += ca
            return "a"
        self.v += cv
        return "v"


def build(nreps=1, agg_fp8=AGG_FP8):
    nc = bacc.Bacc("TRN2", target_bir_lowering=False, debug=False)
    PHASES.clear()

    def _mark(label):
        PHASES.append((label, nc.next_id()))

    d = {}

    def din(name, shape, dt):
        d[name] = nc.dram_tensor(name, list(shape), dt, kind="ExternalInput")

    din("verts", (4, NP), F32R)
    din("wsm", (4, 1536), F32R)            # w0vt (1024) | skv (512)
    din("wcat", (128, WCOLS), F16)
    if agg_fp8:
        din("at8", (128, NT2, 2, NP), FP8)
    else:
        din("atp", (128, NT, NP), F16)
    din("g8n", (128, 16), F32)             # indicator / (8*N)
    din("g8t", (16, 128), F32)             # 0/1 indicator transpose
    din("prm", (128, NSLOT), F32)
    out_d = nc.dram_tensor("out", [BLOC, 3, N], F32, kind="ExternalOutput")

    with tile.TileContext(nc) as tc, contextlib.ExitStack() as ctx:
        cons = ctx.enter_context(tc.tile_pool(name="cons", bufs=1))
        ps = ctx.enter_context(tc.tile_pool(name="ps", bufs=8, space="PSUM"))
        sm = ctx.enter_context(tc.tile_pool(name="sm", bufs=2))
        xrp = ctx.enter_context(tc.tile_pool(name="xrp", bufs=6))
        scp = ctx.enter_context(tc.tile_pool(name="scp", bufs=3))

        # ---- constants (outside the repeat loop) ----
        g8n = cons.tile([128, 16], F32)
        nc.sync.dma_start(g8n[:], d["g8n"].ap())
        g8t = cons.tile([16, 128], F32)
        nc.sync.dma_start(g8t[:], d["g8t"].ap())
        prm = cons.tile([128, NSLOT], F32)
        nc.sync.dma_start(prm[:], d["prm"].ap())
        verts = cons.tile([4, NP], F32R)
        nc.sync.dma_start(verts[:], d["verts"].ap())
        wsm = cons.tile([4, 1536], F32R)
        nc.sync.dma_start(wsm[:], d["wsm"].ap())
        wcat = cons.tile([128, WCOLS], F16)
        hw = WCOLS // 2
        nc.sync.dma_start(wcat[:, 0:hw], d["wcat"].ap()[:, 0:hw])
        nc.gpsimd.dma_start(wcat[:, hw:WCOLS], d["wcat"].ap()[:, hw:WCOLS])
        if agg_fp8:
            at8 = cons.tile([128, NT2, 2, NP], FP8)
            for k2 in range(NT2):
                eng = [nc.sync, nc.gpsimd, nc.scalar][k2 % 3]
                eng.dma_start(at8[:, k2, :, :], d["at8"].ap()[:, k2])
        else:
            asb = cons.tile([128, NT, NP], F16)
            for kt in range(NT):
                eng = [nc.sync, nc.gpsimd, nc.scalar][kt % 3]
                eng.dma_start(asb[:, kt, :], d["atp"].ap()[:, kt])
        eps = cons.tile([16, 1], F32)
        nc.vector.memset(eps[:], 1e-5)

        def W(name, parts=128):
            p0, w = _WC[name]
            return wcat[0:parts, p0:p0 + w]

        def P(name, t=0, parts=128, width=1):
            i = PIDX[(name, t)]
            return prm[0:parts, i:i + width]

        # fixed activation tiles per batch
        bt = []
        for b in range(BLOC):
            st = {
                "x": [cons.tile([128, NP], F16, name=f"x{b}_{m}")
                      for m in range(4)],
                "y1": [cons.tile([128, NP], F16, name=f"y1_{b}_{m}")
                       for m in range(2)],
                "y2": [cons.tile([128, NP], F16, name=f"y2_{b}_{m}")
                       for m in range(2)],
                "yh1": cons.tile([64, NP], F16, name=f"yh1_{b}"),
                "yh2": cons.tile([32, NP], F16, name=f"yh2_{b}"),
                "osb": cons.tile([4, NP], F32, name=f"osb_{b}"),
                "xab": None,
            }
            if agg_fp8:
                st["sup"] = cons.tile([128, NT2, 2, 256], FP8,
                                      name=f"sup{b}")
                st["supr"] = cons.tile([128, NT2, 2, 256], FP8,
                                       name=f"supr{b}")
            else:
                st["sup"] = cons.tile([128, NT, 256], F16, name=f"sup{b}")
            bt.append(st)
        for b in range(BLOC):
            # pad column (vertex 1723) is never written by evacuations;
            # zero it once so matmul reads stay finite and exact
            for m in range(4):
                nc.vector.memset(bt[b]["x"][m][:, N:NP], 0.0)
            for m in range(2):
                nc.vector.memset(bt[b]["y1"][m][:, N:NP], 0.0)
                nc.vector.memset(bt[b]["y2"][m][:, N:NP], 0.0)
            nc.vector.memset(bt[b]["yh1"][:, N:NP], 0.0)
            nc.vector.memset(bt[b]["yh2"][:, N:NP], 0.0)
            nc.vector.memset(bt[b]["osb"][:, N:NP], 0.0)
            if agg_fp8:
                # stale tail rows of the last double-tile (i=1 rows 60..127
                # are never written by sup evacs; fp8 garbage can be NaN)
                nc.vector.memset(bt[b]["sup"][:, NT2 - 1, 1, :], 0.0)
                nc.vector.memset(bt[b]["supr"][:, NT2 - 1, 1, :], 0.0)

        bal = _Bal()

        # ---------- emission helpers ----------
        def evac(dst, src, bias, stt=None, slot=0, residual=None, w=432,
                 parts=128):
            """dst = src + bias (+ residual), optional accum into stt[:,slot].
            src is PSUM f32; dst SBUF."""
            acc = stt[0:parts, slot:slot + 1] if stt is not None else None
            if residual is not None:
                bal.v += 1.042 * w + 200
                nc.vector.scalar_tensor_tensor(
                    dst, src, bias, residual, op0=ALU.add, op1=ALU.add,
                    accum_out=acc)
                return
            e = bal.pick(0.833 * w + 250, 1.042 * w + 200)
            if e == "a":
                nc.scalar.activation(dst, src, AF.Identity, bias=bias,
                                     accum_out=acc)
            elif acc is not None:
                nc.vector.tensor_scalar(dst, src, bias, 0.0, op0=ALU.add,
                                        op1=ALU.add, accum_out=acc)
            else:
                nc.vector.tensor_scalar(dst, src, bias, None, op0=ALU.add)

        def relu_evac(dst, src, bias, w=432, parts=128):
            e = bal.pick(0.833 * w + 250, 2 * (0.52 * w + 130))
            if e == "a":
                nc.scalar.activation(dst, src, AF.Relu, bias=bias)
            else:
                nc.vector.tensor_scalar(dst, src, bias, None, op0=ALU.add)
                nc.vector.tensor_scalar_max(dst, dst, 0.0)

        def relu_scale_chunk(dst, src, a_ap, b_ap, w):
            e = bal.pick(0.833 * w + 250, 2 * (0.45 * w + 130))
            if e == "a":
                nc.scalar.activation(dst, src, AF.Relu, bias=b_ap,
                                     scale=a_ap)
            else:
                nc.vector.tensor_scalar(dst, src, a_ap, b_ap, op0=ALU.mult,
                                        op1=ALU.add)
                nc.vector.tensor_scalar_max(dst, dst, 0.0)

        def relu_apply(row_ap_fn, a_ap, b_ap, parts=128):
            """In-place y = relu(a*y + b) over a full row, split main/tail."""
            for (f0, fw) in ((0, ROWM), (ROWM, N - ROWM)):
                ap = row_ap_fn(f0, fw)
                e = bal.pick(0.833 * fw + 250, 2 * (0.3 * fw + 130))
                if e == "a":
                    nc.scalar.activation(ap, ap, AF.Relu, bias=b_ap,
                                         scale=a_ap)
                else:
                    nc.vector.tensor_scalar(ap, ap, a_ap, b_ap, op0=ALU.mult,
                                            op1=ALU.add)
                    nc.vector.tensor_scalar_max(ap, ap, 0.0)

        def square_stats(row_ap_fn, stt, parts=128):
            """Accumulate sum(x^2) of a row into stt slots 4,5 (pad excl)."""
            for j, (f0, fw) in enumerate(((0, ROWM), (ROWM, N - ROWM))):
                ap = row_ap_fn(f0, fw)
                scr = scp.tile([128, ROWM], F16, tag="scr", bufs=3,
                               name="scr")
                e = bal.pick(0.833 * fw + 250, 0.3 * fw + 130)
                acc = stt[0:parts, 4 + j:5 + j]
                if e == "a":
                    nc.scalar.activation(scr[0:parts, 0:fw], ap, AF.Square,
                                         accum_out=acc)
                else:
                    nc.vector.scalar_tensor_tensor(
                        scr[0:parts, 0:fw], ap, 1.0, ap, op0=ALU.mult,
                        op1=ALU.mult, accum_out=acc)

        def new_st(tag="st", n=1):
            return [sm.tile([128, 8], F32, tag=tag, bufs=10, name="st")
                    for _ in range(n)]

        def gn_chain(sts, gname, bname, parts=128, G=16, abtag="ab",
                     abbufs=4, gt0=0):
            """Batched GN chain over T=len(sts) channel tiles.
            sts[t] holds [sum0,sum1,sum2,sum3, sq0,sq1] per channel.
            Returns ab [parts, T, 2] with per-channel [a, beta]."""
            T = len(sts)
            stc = sm.tile([128, 8, 6], F32, tag="stc", bufs=4, name="stc")
            for t, stt in enumerate(sts):
                nc.vector.tensor_copy(stc[0:parts, t, :], stt[0:parts, 0:6])
            psg = ps.tile([16, 8, 6], F32, tag="ps", name="psg")
            nc.tensor.matmul(psg[0:G, 0:T, :], g8n[0:parts, 0:G],
                             stc[0:parts, 0:T, :], start=True, stop=True)
            mr = sm.tile([16, 8, 2], F32, tag="mr", bufs=4, name="mr")
            nc.vector.tensor_reduce(mr[0:G, 0:T, 0:1], psg[0:G, 0:T, 0:4],
                                    mybir.AxisListType.X, ALU.add)
            e2 = sm.tile([16, 8], F32, tag="e2", bufs=4, name="e2")
            nc.vector.tensor_reduce(e2[0:G, 0:T].unsqueeze(-1),
                                    psg[0:G, 0:T, 4:6],
                                    mybir.AxisListType.X, ALU.add)
            sq = sm.tile([16, 8], F32, tag="sq", bufs=4, name="sq")
            nc.vector.tensor_tensor(sq[0:G, 0:T], mr[0:G, 0:T, 0],
                                    mr[0:G, 0:T, 0], op=ALU.mult)
            nc.vector.tensor_tensor(e2[0:G, 0:T], e2[0:G, 0:T], sq[0:G, 0:T],
                                    op=ALU.subtract)
            nc.scalar.activation(e2[0:G, 0:T], e2[0:G, 0:T], AF.Sqrt,
                                 bias=eps[0:G, :])
            nc.vector.reciprocal(mr[0:G, 0:T, 1], e2[0:G, 0:T])
            psb = ps.tile([128, 8, 2], F32, tag="ps", name="psb")
            nc.tensor.matmul(psb[0:parts, 0:T, :], g8t[0:G, 0:parts],
                             mr[0:G, 0:T, :], start=True, stop=True)
            ab = sm.tile([128, 8, 2], F32, tag=abtag, bufs=abbufs, name="ab")
            gv = sm.tile([128, 8], F32, tag="gv", bufs=4, name="gv")
            for t in range(T):
                nc.vector.tensor_copy(gv[0:parts, t:t + 1],
                                      P(gname, gt0 + t, parts))
            nc.vector.tensor_tensor(ab[0:parts, 0:T, 0], psb[0:parts, 0:T, 1],
                                    gv[0:parts, 0:T], op=ALU.mult)
            t3 = sm.tile([128, 8], F32, tag="t3", bufs=4, name="t3")
            nc.vector.tensor_tensor(t3[0:parts, 0:T], psb[0:parts, 0:T, 0],
                                    ab[0:parts, 0:T, 0], op=ALU.mult)
            for t in range(T):
                nc.vector.tensor_copy(gv[0:parts, t:t + 1],
                                      P(bname, gt0 + t, parts))
            nc.vector.tensor_tensor(ab[0:parts, 0:T, 1], gv[0:parts, 0:T],
                                    t3[0:parts, 0:T], op=ALU.subtract)
            return ab

        # ---------- phases ----------
        def b0front(S, b):
            _mark("b0front")
            y1 = S["y1"]
            x = S["x"]
            y1st = new_st(n=2)
            xst = None
            for ci, (f0, fw) in enumerate(MCH):
                y1ps = [ps.tile([128, 512], F32, tag="ps", name="y1ps")
                        for _ in range(2)]
                for kt in range(8):
                    ups = ps.tile([128, 512], F32, tag="ps", name="ups")
                    nc.tensor.matmul(ups[:, :fw],
                                     wsm[:, kt * 128:(kt + 1) * 128],
                                     verts[:, f0:f0 + fw],
                                     start=True, stop=True)
                    xr = xrp.tile([128, 432], F16, tag="xr8", bufs=4,
                                  name="x0r")
                    relu_scale_chunk(xr[:, :fw], ups[:, :fw],
                                     P(f"ab0a{b}", kt), P(f"ab0b{b}", kt), fw)
                    for mt in range(2):
                        nc.tensor.matmul(y1ps[mt][:, :fw],
                                         W(f"b0l1_{kt}")[:, mt * 128:
                                                         (mt + 1) * 128],
                                         xr[:, :fw],
                                         start=(kt == 0), stop=(kt == 7))
                for mt in range(4):
                    skps = ps.tile([128, 512], F32, tag="ps", name="skps")
                    nc.tensor.matmul(skps[:, :fw],
                                     wsm[:, 1024 + mt * 128:1024 +
                                         (mt + 1) * 128],
                                     verts[:, f0:f0 + fw], start=True,
                                     stop=True)
                    rw = fw if f0 + fw <= N else (N - f0)
                    evac(x[mt][:, f0:f0 + rw], skps[:, :rw],
                         P(f"skb{b}", mt), w=rw)
                for mt in range(2):
                    rw = fw if f0 + fw <= N else (N - f0)
                    evac(y1[mt][:, f0:f0 + rw], y1ps[mt][:, :rw],
                         P("b0_lin1_b", mt), stt=y1st[mt], slot=ci, w=rw)
            for mt in range(2):
                square_stats(lambda f0, fwx, m=mt: y1[m][:, f0:f0 + fwx],
                             y1st[mt])
            S["y1st"] = y1st

        def lin1(S, i):
            _mark("lin1")
            y1 = S["y1"]
            x = S["x"]
            abx = S["xab"]
            y1st = new_st(n=2)
            for ci, (f0, fw) in enumerate(MCH):
                y1ps = [ps.tile([128, 512], F32, tag="ps", name="y1psb")
                        for _ in range(2)]
                for ct in range(4):
                    xr = xrp.tile([128, 432], F16, tag="xr", bufs=4,
                                  name="xrb")
                    relu_scale_chunk(xr[:, :fw], x[ct][:, f0:f0 + fw],
                                     abx[:, ct, 0:1], abx[:, ct, 1:2], fw)
                    for mt in range(2):
                        nc.tensor.matmul(
                            y1ps[mt][:, :fw],
                            W(f"bl1_{i}_{ct}")[:, mt * 128:(mt + 1) * 128],
                            xr[:, :fw], start=(ct == 0), stop=(ct == 3))
                for mt in range(2):
                    rw = fw if f0 + fw <= N else (N - f0)
                    evac(y1[mt][:, f0:f0 + rw], y1ps[mt][:, :rw],
                         P(f"lin1_b{i}", mt), stt=y1st[mt], slot=ci, w=rw)
            for mt in range(2):
                square_stats(lambda f0, fwx, m=mt: y1[m][:, f0:f0 + fwx],
                             y1st[mt])
            S["y1st"] = y1st

        def sup_phase(S, pn_n1g, pn_n1b, cwn):
            _mark("sup")
            y1 = S["y1"]
            sup = S["sup"]
            supr = S.get("supr")
            ab = gn_chain(S["y1st"], pn_n1g, pn_n1b, abtag="aby", abbufs=4)
            for mt in range(2):
                relu_apply(lambda f0, fwx, m=mt: y1[m][:, f0:f0 + fwx],
                           ab[:, mt, 0:1], ab[:, mt, 1:2])
            for half in (range(0, 5), range(5, 10), range(10, NT)):
                spss = {}
                for ct in range(2):
                    for nt in half:
                        ms = nt * 128
                        mw = min(ms + 128, NP) - ms
                        if ct == 0:
                            spss[nt] = ps.tile([128, 256], F32, tag="ps",
                                               name="sps")
                        nc.tensor.matmul(spss[nt][0:mw, :],
                                         y1[ct][:, ms:ms + mw],
                                         W(f"{cwn}_{ct}"),
                                         start=(ct == 0), stop=(ct == 1))
                for nt in half:
                    ms = nt * 128
                    mw = min(ms + 128, NP) - ms
                    if agg_fp8:
                        dst = sup[0:mw, nt // 2, nt % 2, :]
                        e = bal.pick(0.833 * 256 + 250, 1.042 * 256 + 200)
                        if e == "a":
                            nc.scalar.activation(dst, spss[nt][0:mw, :],
                                                 AF.Copy, scale=SUPS)
                        else:
                            nc.vector.tensor_scalar(dst, spss[nt][0:mw, :],
                                                    SUPS, None, op0=ALU.mult)
                        # error-feedback residual r = f8(s*sup - q1)
                        bal.v += 1.042 * 256 + 200
                        nc.vector.scalar_tensor_tensor(
                            supr[0:mw, nt // 2, nt % 2, :],
                            spss[nt][0:mw, :], SUPS, dst,
                            op0=ALU.mult, op1=ALU.subtract)
                    else:
                        dst = sup[0:mw, nt, :]
                        e = bal.pick(0.833 * 256 + 250, 1.042 * 256 + 200)
                        if e == "a":
                            nc.scalar.activation(dst, spss[nt][0:mw, :],
                                                 AF.Copy)
                        else:
                            nc.vector.tensor_copy(dst, spss[nt][0:mw, :])

        def agg_phase(S, pn_cb, pn_n2g, pn_n2b):
            _mark("agg")
            y2 = S["y2"]
            sup = S["sup"]
            supr = S.get("supr")
            y2st = new_st(n=2)
            scale = (1.0 / SUPS) if agg_fp8 else 1.0
            for dt in range(2):
                for ci, (f0, fw) in enumerate(MCH):
                    aps = ps.tile([128, 512], F32, tag="ps", name="aps")
                    if agg_fp8:
                        for gi, sp in enumerate((sup, supr)):
                            for k2 in range(NT2):
                                nc.tensor.matmul(
                                    aps[:, :fw],
                                    sp[:, k2, :, dt * 128:(dt + 1) * 128],
                                    at8[:, k2, :, f0:f0 + fw],
                                    start=(gi == 0 and k2 == 0),
                                    stop=(gi == 1 and k2 == NT2 - 1),
                                    perf_mode=mybir.MatmulPerfMode.DoubleRow)
                    else:
                        for kt in range(NT):
                            kn = min(128, N - kt * 128)
                            nc.tensor.matmul(
                                aps[:, :fw],
                                sup[0:kn, kt, dt * 128:(dt + 1) * 128],
                                asb[0:kn, kt, f0:f0 + fw],
                                start=(kt == 0), stop=(kt == NT - 1))
                    rw = fw if f0 + fw <= N else (N - f0)
                    # evac with bias and 1/SUPS scale
                    acc = y2st[dt][0:128, ci:ci + 1]
                    e = bal.pick(0.833 * rw + 250, 1.042 * rw + 200)
                    if e == "a" or not agg_fp8:
                        nc.scalar.activation(y2[dt][:, f0:f0 + rw],
                                             aps[:, :rw], AF.Identity,
                                             bias=P(pn_cb, dt), scale=scale,
                                             accum_out=acc)
                    else:
                        nc.vector.tensor_scalar(y2[dt][:, f0:f0 + rw],
                                                aps[:, :rw], scale,
                                                P(pn_cb, dt), op0=ALU.mult,
                                                op1=ALU.add, accum_out=acc)
                square_stats(lambda f0, fwx, m=dt: y2[m][:, f0:f0 + fwx],
                             y2st[dt])
                _mark("gn3")
                ab = gn_chain([y2st[dt]], pn_n2g, pn_n2b, abtag="aby",
                              abbufs=4, gt0=dt)
                relu_apply(lambda f0, fwx, m=dt: y2[m][:, f0:f0 + fwx],
                           ab[:, 0, 0:1], ab[:, 0, 1:2])

        def lin2_phase(S, l2n, pn_l2b, pn_gnext, collect):
            _mark("lin2")
            x = S["x"]
            y2 = S["y2"]
            xst = new_st(tag="stx", n=4) if collect else None
            for ci, (f0, fw) in enumerate(MCH):
                rw = fw if f0 + fw <= N else (N - f0)
                for mt in range(4):
                    lps = ps.tile([128, 512], F32, tag="ps", name="lps")
                    for ct in range(2):
                        nc.tensor.matmul(
                            lps[:, :fw],
                            W(f"{l2n}_{ct}")[:, mt * 128:(mt + 1) * 128],
                            y2[ct][:, f0:f0 + fw],
                            start=(ct == 0), stop=(ct == 1))
                    evac(x[mt][:, f0:f0 + rw], lps[:, :rw], P(pn_l2b, mt),
                         stt=xst[mt] if collect else None, slot=ci,
                         residual=x[mt][:, f0:f0 + rw], w=rw)
            if collect:
                for mt in range(4):
                    square_stats(lambda f0, fwx, m=mt: x[m][:, f0:f0 + fwx],
                                 xst[mt])
                _mark("gnx")
                g, bn = pn_gnext
                S["xab"] = gn_chain(xst, g, bn, abtag="abx", abbufs=2)
            else:
                S["xab"] = None

        def head_phase(S, b):
            _mark("head")
            x = S["x"]
            yh1, yh2, osb = S["yh1"], S["yh2"], S["osb"]
            for (f0, fw) in MCH:
                rw = fw if f0 + fw <= N else (N - f0)
                hps = ps.tile([64, 512], F32, tag="ps", name="hps")
                for kt in range(4):
                    nc.tensor.matmul(hps[:, :fw], W(f"h1_{kt}"),
                                     x[kt][:, f0:f0 + fw],
                                     start=(kt == 0), stop=(kt == 3))
                relu_evac(yh1[:, f0:f0 + rw], hps[0:64, :rw],
                          P("h1_b", 0, 64), w=rw, parts=64)
            hst = new_st(n=1)
            for ci, (f0, fw) in enumerate(MCH):
                rw = fw if f0 + fw <= N else (N - f0)
                hps2 = ps.tile([32, 512], F32, tag="ps", name="hps2")
                nc.tensor.matmul(hps2[:, :fw], W("h2", 64), yh1[:, f0:f0 + fw],
                                 start=True, stop=True)
                evac(yh2[:, f0:f0 + rw], hps2[0:32, :rw], P("h2_b", 0, 32),
                     stt=hst[0], slot=ci, w=rw, parts=32)
            square_stats(lambda f0, fwx: yh2[:, f0:f0 + fwx], hst[0],
                         parts=32)
            abh = gn_chain(hst, "hn_g", "hn_b", parts=32, G=4, abtag="abh")
            relu_apply(lambda f0, fwx: yh2[:, f0:f0 + fwx],
                       abh[0:32, 0, 0:1], abh[0:32, 0, 1:2], parts=32)
            for (f0, fw) in MCH:
                rw = fw if f0 + fw <= N else (N - f0)
                hps3 = ps.tile([4, 512], F32, tag="ps", name="hps3")
                nc.tensor.matmul(hps3[:, :fw], W("h3", 32), yh2[:, f0:f0 + fw],
                                 start=True, stop=True)
                evac(osb[0:3, f0:f0 + rw], hps3[0:3, :rw], P("h3_b", 0, 3),
                     w=rw, parts=3)
            nc.sync.dma_start(out_d.ap()[b], osb[0:3, 0:N])

        rep = tc.For_i(0, nreps, 1) if nreps > 1 else contextlib.nullcontext()
        with rep:
            for b in range(BLOC):
                b0front(bt[b], b)
            for b in range(BLOC):
                sup_phase(bt[b], "b0_n1_g", "b0_n1_b", "b0cw")
            for b in range(BLOC):
                agg_phase(bt[b], "b0_conv_b", "b0_n2_g", "b0_n2_b")
            for b in range(BLOC):
                lin2_phase(bt[b], "b0l2", "b0_lin2_b",
                           ("blk0_pre_g", "blk0_pre_b"), True)
            for i in range(L):
                for b in range(BLOC):
                    lin1(bt[b], i)
                for b in range(BLOC):
                    sup_phase(bt[b], f"n1_g{i}", f"n1_b{i}", f"bcw_{i}")
                for b in range(BLOC):
                    agg_phase(bt[b], f"conv_b{i}", f"n2_g{i}", f"n2_b{i}")
                for b in range(BLOC):
                    collect = i < L - 1
                    nxt = (f"pre_g{i + 1}", f"pre_b{i + 1}") if collect \
                        else (None, None)
                    lin2_phase(bt[b], f"bl2_{i}", f"lin2_b{i}", nxt, collect)
            for b in range(BLOC):
                head_phase(bt[b], b)

    nc.compile()
    return nc


def _host_prep(inputs, agg_fp8=AGG_FP8):
    f32 = np.float32
    shared = {}

    verts_in = np.asarray(inputs["ref_vertices"], f32)
    verts = np.zeros((4, NP), f32)
    verts[0:3, 0:N] = verts_in
    shared["verts"] = verts

    lin0_W = np.asarray(inputs["lin0_W"], f32)
    w0v = lin0_W[:, 0:3]                       # (1024, 3)
    skW = np.asarray(inputs["b0_skip_W"], f32)  # (512, 1024)
    wsm = np.zeros((4, 1536), f32)
    wsm[0:3, 0:1024] = w0v.T
    wsm[0:3, 1024:1536] = (skW @ w0v).T        # skv
    shared["wsm"] = wsm

    # adjacency
    src = np.asarray(inputs["adj_src"]).astype(np.int64)
    dst = np.asarray(inputs["adj_dst"]).astype(np.int64)
    w = np.asarray(inputs["adj_w"], f32)
    at = np.zeros((NT * 128, NP), f32)
    np.add.at(at, (src, dst), w)
    if agg_fp8:
        a8 = at.reshape(NT2, 2, 128, NP).transpose(2, 0, 1, 3)
        shared["at8"] = np.ascontiguousarray(a8).astype(
            mybir.dt.np(FP8))
    else:
        atp = at.reshape(NT, 128, NP).transpose(1, 0, 2)
        shared["atp"] = np.ascontiguousarray(atp).astype(np.float16)

    # weights concat (bf16)
    wcat = np.zeros((128, WCOLS), f32)

    def put(name, arr):
        p0, wd = _WC[name]
        pr, wc = arr.shape
        assert wc == wd, (name, arr.shape)
        wcat[0:pr, p0:p0 + wd] = arr

    b0l1 = np.asarray(inputs["b0_lin1_W"], f32).T      # (1024, 256)
    for kt in range(8):
        put(f"b0l1_{kt}", b0l1[kt * 128:(kt + 1) * 128])
    b0cw = np.asarray(inputs["b0_conv_W"], f32)        # (256, 256)
    put("b0cw_0", b0cw[0:128]); put("b0cw_1", b0cw[128:256])
    b0l2 = np.asarray(inputs["b0_lin2_W"], f32).T      # (256, 512)
    put("b0l2_0", b0l2[0:128]); put("b0l2_1", b0l2[128:256])
    bl1 = np.asarray(inputs["blk_lin1_W"], f32)        # (L, 256, 512)
    bcw = np.asarray(inputs["blk_conv_W"], f32)        # (L, 256, 256)
    bl2 = np.asarray(inputs["blk_lin2_W"], f32)        # (L, 512, 256)
    for i in range(L):
        t1 = bl1[i].T                                  # (512, 256)
        for ct in range(4):
            put(f"bl1_{i}_{ct}", t1[ct * 128:(ct + 1) * 128])
        put(f"bcw_{i}_0", bcw[i][0:128]); put(f"bcw_{i}_1", bcw[i][128:256])
        t2 = bl2[i].T                                  # (256, 512)
        put(f"bl2_{i}_0", t2[0:128]); put(f"bl2_{i}_1", t2[128:256])
    h1 = np.asarray(inputs["h1_W"], f32).T             # (512, 64)
    for kt in range(4):
        put(f"h1_{kt}", h1[kt * 128:(kt + 1) * 128])
    put("h2", np.asarray(inputs["h2_W"], f32).T)       # (64, 32)
    h3 = np.zeros((32, 4), f32)
    h3[:, 0:3] = np.asarray(inputs["h3_W"], f32).T
    put("h3", h3)
    shared["wcat"] = wcat.astype(np.float16)

    ind = np.zeros((128, 16), f32)
    for c in range(128):
        ind[c, c // 8] = 1.0
    shared["g8n"] = ind / (8.0 * N)
    shared["g8t"] = np.ascontiguousarray(ind.T)

    # host-side lin0 / b0-GN1 analytics
    img = np.asarray(inputs["image_resnet"], f32)      # (B, 2048)
    vb_all = lin0_W[:, 3:] @ img.T + \
        np.asarray(inputs["lin0_b"], f32)[:, None]     # (1024, B)
    U = lin0_W[:, 0:3] @ verts_in                      # (1024, N)
    mU = U.mean(1)
    e2U = (U * U).mean(1)
    pre_g = np.asarray(inputs["b0_pre_g"], f32)
    pre_b = np.asarray(inputs["b0_pre_b"], f32)
    skb = np.asarray(inputs["b0_skip_b"], f32)

    vals = {"b0_lin1_b": inputs["b0_lin1_b"],
            "b0_n1_g": inputs["b0_n1_g"], "b0_n1_b": inputs["b0_n1_b"],
            "b0_conv_b": inputs["b0_conv_b"],
            "b0_n2_g": inputs["b0_n2_g"], "b0_n2_b": inputs["b0_n2_b"],
            "b0_lin2_b": inputs["b0_lin2_b"],
            "blk0_pre_g": np.asarray(inputs["blk_pre_g"])[0],
            "blk0_pre_b": np.asarray(inputs["blk_pre_b"])[0],
            "h1_b": inputs["h1_b"], "h2_b": inputs["h2_b"],
            "hn_g": inputs["hn_g"], "hn_b": inputs["hn_b"],
            "h3_b": inputs["h3_b"]}
    for i in range(L):
        vals[f"lin1_b{i}"] = np.asarray(inputs["blk_lin1_b"])[i]
        vals[f"n1_g{i}"] = np.asarray(inputs["blk_n1_g"])[i]
        vals[f"n1_b{i}"] = np.asarray(inputs["blk_n1_b"])[i]
        vals[f"conv_b{i}"] = np.asarray(inputs["blk_conv_b"])[i]
        vals[f"n2_g{i}"] = np.asarray(inputs["blk_n2_g"])[i]
        vals[f"n2_b{i}"] = np.asarray(inputs["blk_n2_b"])[i]
        vals[f"lin2_b{i}"] = np.asarray(inputs["blk_lin2_b"])[i]
        if i < L - 1:
            vals[f"pre_g{i + 1}"] = np.asarray(inputs["blk_pre_g"])[i + 1]
            vals[f"pre_b{i + 1}"] = np.asarray(inputs["blk_pre_b"])[i + 1]

    in_maps = []
    for core in range(NCORES):
        vals_c = dict(vals)
        for bl in range(BLOC):
            bglob = core * BLOC + bl
            vb = vb_all[:, bglob]                      # (1024,)
            m_c = mU + vb
            e2_c = e2U + 2 * mU * vb + vb * vb
            m_g = m_c.reshape(128, 8).mean(1)
            e2_g = e2_c.reshape(128, 8).mean(1)
            var_g = e2_g - m_g * m_g
            rs = 1.0 / np.sqrt(var_g + 1e-5)
            a_c = pre_g * np.repeat(rs, 8)
            b_c = pre_b - np.repeat(m_g * rs, 8) * pre_g
            vals_c[f"ab0a{bl}"] = a_c
            vals_c[f"ab0b{bl}"] = b_c + a_c * vb
            vals_c[f"skb{bl}"] = skW @ vb + skb
        prm = np.zeros((128, NSLOT), f32)
        for (name, t), pos in PIDX.items():
            vec = np.asarray(vals_c[name], f32).ravel()
            seg = vec[t * 128:(t + 1) * 128]
            prm[0:len(seg), pos] = seg
        m = dict(shared)
        m["prm"] = prm
        in_maps.append(m)
    return in_maps


_NC_CACHE = {}


def _get_nc(nreps=1, **kw):
    key = (nreps, tuple(sorted(kw.items())))
    if key not in _NC_CACHE:
        _NC_CACHE[key] = build(nreps, **kw)
    return _NC_CACHE[key]


def run_on_hw(inputs, nreps=1, **kw):
    nc = _get_nc(nreps, **kw)
    in_maps = _host_prep(inputs, agg_fp8=kw.get("agg_fp8", AGG_FP8))
    res = run_bass_kernel_spmd(nc, in_maps, core_ids=list(range(NCORES)),
                               trace=False)
    return np.concatenate([res.results[c]["out"] for c in range(NCORES)],
                          axis=0)


def kernel(**inputs) -> np.ndarray:
    return run_on_hw(inputs, nreps=1)


# revision 17
# speedup vs baseline: 1.2051x; 1.2051x over previous
"""Trainium2 Bass kernel for nn_ClothGraphConvNetwork_MLPDecoder.

8 NeuronCores, data-parallel over batch (2 batches/core), no collectives.

v2 design (vs v1 baseline ~950us/iter):
- Host precomputes everything that depends only on (weights, image): the
  per-batch lin0 image projection v_b, the analytic b0 GroupNorm1
  coefficients (vb folded into the bias), the rank-4 skip projection
  skv = skW @ W0v^T and its per-batch bias.  The 2048-wide lin0/skip
  matmuls never run on device.
- All weights SBUF-resident in bf16, loaded once outside the repeat
  loop in a handful of large DMAs.
- Activations bf16 (PSUM accumulation f32).  GroupNorm statistics come
  from accum_out sums fused into the PSUM evacuation instructions plus
  one Square pass per row; bn_stats is gone.
- Residual adds fused into the lin2 evacuation (DVE scalar_tensor_tensor
  psum+bias+x_old with accum), replacing the identity matmuls.
- Elementwise work split between Activation and DVE engines by a greedy
  load balancer; Pool only issues DMAs.
- The two local batches are emitted interleaved phase-by-phase so each
  batch's GroupNorm chain latency hides under the other batch's matmuls.
- Graph aggregation: dense adjacency matmul.  AGG_FP8 uses fp8e4m3 with
  DoubleRow perf mode and a two-pass error-feedback split
  (sup*s = q1 + r, both fp8) so the quantization error is ~fp8^2;
  otherwise plain f16.
"""

import contextlib

import numpy as np
import ml_dtypes

import concourse.bass as bass
import concourse.tile as tile
from concourse import bacc, mybir
from concourse.bass_utils import run_bass_kernel_spmd

F32R = mybir.dt.float32r
F32 = mybir.dt.float32
F16 = mybir.dt.float16
FP8 = mybir.dt.float8e4
AF = mybir.ActivationFunctionType
ALU = mybir.AluOpType

B, N, DEG = 16, 1723, 8
C, L, H = 512, 5, 256
NP = 1724
NCORES = 8
BLOC = B // NCORES
NT = 14                 # 128-vertex tiles
NT2 = 7                 # 256-vertex double tiles (fp8 DoubleRow)
MCH = [(0, 432), (432, 432), (864, 432), (1296, 428)]   # matmul chunks
ROWM = 1296             # full-row main piece; tail is [1296:1723]
SUPS = 64.0             # fp8 sup scale
AGG_FP8 = True

# wcat (bf16 weights) column layout
_WC = {}
_pos = 0
def _wslot(name, w):
    global _pos
    _WC[name] = (_pos, w)
    _pos += w
for _kt in range(8):
    _wslot(f"b0l1_{_kt}", H)
_wslot("b0cw_0", H); _wslot("b0cw_1", H)
_wslot("b0l2_0", C); _wslot("b0l2_1", C)
for _i in range(L):
    for _ct in range(4):
        _wslot(f"bl1_{_i}_{_ct}", H)
    _wslot(f"bcw_{_i}_0", H); _wslot(f"bcw_{_i}_1", H)
    _wslot(f"bl2_{_i}_0", C); _wslot(f"bl2_{_i}_1", C)
for _kt in range(4):
    _wslot(f"h1_{_kt}", 64)
_wslot("h2", 32)
_wslot("h3", 4)
WCOLS = _pos


def _param_layout():
    items = [("b0_lin1_b", 256), ("b0_n1_g", 256), ("b0_n1_b", 256),
             ("b0_conv_b", 256), ("b0_n2_g", 256), ("b0_n2_b", 256),
             ("b0_lin2_b", 512),
             ("blk0_pre_g", 512), ("blk0_pre_b", 512)]
    for i in range(L):
        items += [(f"lin1_b{i}", 256), (f"n1_g{i}", 256), (f"n1_b{i}", 256),
                  (f"conv_b{i}", 256), (f"n2_g{i}", 256), (f"n2_b{i}", 256),
                  (f"lin2_b{i}", 512)]
        if i < L - 1:
            items += [(f"pre_g{i + 1}", 512), (f"pre_b{i + 1}", 512)]
    items += [("h1_b", 64), ("h2_b", 32), ("hn_g", 32), ("hn_b", 32),
              ("h3_b", 3)]
    for b in range(BLOC):
        items += [(f"ab0a{b}", 1024), (f"ab0b{b}", 1024), (f"skb{b}", 512)]
    idx = {}
    pos = 0
    for name, ln in items:
        for t in range((ln + 127) // 128):
            idx[(name, t)] = pos
            pos += 1
    return items, idx, pos


PARAM_ITEMS, PIDX, NSLOT = _param_layout()
PHASES = []


class _Bal:
    """Greedy Act/DVE load balancer (ns units)."""

    def __init__(self):
        self.a = 0.0
        self.v = 0.0

    def pick(self, ca, cv):
        if self.a + ca <= self.v + cv:
            self.a += ca
            return "a"
        self.v += cv
        return "v"


def build(nreps=1, agg_fp8=AGG_FP8):
    nc = bacc.Bacc("TRN2", target_bir_lowering=False, debug=False)
    PHASES.clear()

    def _mark(label):
        PHASES.append((label, nc.next_id()))

    d = {}

    def din(name, shape, dt):
        d[name] = nc.dram_tensor(name, list(shape), dt, kind="ExternalInput")

    din("verts", (4, NP), F32R)
    din("wsm", (4, 1536), F32R)            # w0vt (1024) | skv (512)
    din("wcat", (128, WCOLS), F16)
    if agg_fp8:
        din("at8", (128, NT2, 2, NP), FP8)
    else:
        din("atp", (128, NT, NP), F16)
    din("g8n", (128, 16), F32)             # indicator / (8*N)
    din("g8t", (16, 128), F32)             # 0/1 indicator transpose
    din("prm", (128, NSLOT), F32)
    out_d = nc.dram_tensor("out", [BLOC, 3, N], F32, kind="ExternalOutput")

    with tile.TileContext(nc) as tc, contextlib.ExitStack() as ctx:
        cons = ctx.enter_context(tc.tile_pool(name="cons", bufs=1))
        ps = ctx.enter_context(tc.tile_pool(name="ps", bufs=8, space="PSUM"))
        sm = ctx.enter_context(tc.tile_pool(name="sm", bufs=2))
        xrp = ctx.enter_context(tc.tile_pool(name="xrp", bufs=6))
        scp = ctx.enter_context(tc.tile_pool(name="scp", bufs=3))

        # ---- constants (outside the repeat loop) ----
        g8n = cons.tile([128, 16], F32)
        nc.sync.dma_start(g8n[:], d["g8n"].ap())
        g8t = cons.tile([16, 128], F32)
        nc.sync.dma_start(g8t[:], d["g8t"].ap())
        prm = cons.tile([128, NSLOT], F32)
        nc.sync.dma_start(prm[:], d["prm"].ap())
        verts = cons.tile([4, NP], F32R)
        nc.sync.dma_start(verts[:], d["verts"].ap())
        wsm = cons.tile([4, 1536], F32R)
        nc.sync.dma_start(wsm[:], d["wsm"].ap())
        wcat = cons.tile([128, WCOLS], F16)
        hw = WCOLS // 2
        nc.sync.dma_start(wcat[:, 0:hw], d["wcat"].ap()[:, 0:hw])
        nc.gpsimd.dma_start(wcat[:, hw:WCOLS], d["wcat"].ap()[:, hw:WCOLS])
        if agg_fp8:
            at8 = cons.tile([128, NT2, 2, NP], FP8)
            for k2 in range(NT2):
                eng = [nc.sync, nc.gpsimd, nc.scalar][k2 % 3]
                eng.dma_start(at8[:, k2, :, :], d["at8"].ap()[:, k2])
        else:
            asb = cons.tile([128, NT, NP], F16)
            for kt in range(NT):
                eng = [nc.sync, nc.gpsimd, nc.scalar][kt % 3]
                eng.dma_start(asb[:, kt, :], d["atp"].ap()[:, kt])
        eps = cons.tile([16, 1], F32)
        nc.vector.memset(eps[:], 1e-5)

        def W(name, parts=128):
            p0, w = _WC[name]
            return wcat[0:parts, p0:p0 + w]

        def P(name, t=0, parts=128, width=1):
            i = PIDX[(name, t)]
            return prm[0:parts, i:i + width]

        # fixed activation tiles per batch
        bt = []
        for b in range(BLOC):
            st = {
                "x": [cons.tile([128, NP], F16, name=f"x{b}_{m}")
                      for m in range(4)],
                "y1": [cons.tile([128, NP], F16, name=f"y1_{b}_{m}")
                       for m in range(2)],
                "y2": [cons.tile([128, NP], F16, name=f"y2_{b}_{m}")
                       for m in range(2)],
                "yh1": cons.tile([64, NP], F16, name=f"yh1_{b}"),
                "yh2": cons.tile([32, NP], F16, name=f"yh2_{b}"),
                "osb": cons.tile([4, NP], F32, name=f"osb_{b}"),
                "xab": None,
            }
            if agg_fp8:
                st["sup"] = cons.tile([128, NT, 256], FP8, name=f"sup{b}")
                st["supr"] = cons.tile([128, NT, 256], FP8,
                                       name=f"supr{b}")
            else:
                st["sup"] = cons.tile([128, NT, 256], F16, name=f"sup{b}")
            bt.append(st)
        for b in range(BLOC):
            # pad column (vertex 1723) is never written by evacuations;
            # zero it once so matmul reads stay finite and exact
            for m in range(4):
                nc.vector.memset(bt[b]["x"][m][:, N:NP], 0.0)
            for m in range(2):
                nc.vector.memset(bt[b]["y1"][m][:, N:NP], 0.0)
                nc.vector.memset(bt[b]["y2"][m][:, N:NP], 0.0)
            nc.vector.memset(bt[b]["yh1"][:, N:NP], 0.0)
            nc.vector.memset(bt[b]["yh2"][:, N:NP], 0.0)
            nc.vector.memset(bt[b]["osb"][:, N:NP], 0.0)
            if agg_fp8:
                # stale tail rows of the last double-tile (i=1 rows 60..127
                # are never written by sup evacs; fp8 garbage can be NaN)
                nc.vector.memset(bt[b]["sup"][:, NT - 1, :], 0.0)
                nc.vector.memset(bt[b]["supr"][:, NT - 1, :], 0.0)

        bal = _Bal()

        # ---------- emission helpers ----------
        def evac(dst, src, bias, stt=None, slot=0, residual=None, w=432,
                 parts=128):
            """dst = src + bias (+ residual), optional accum into stt[:,slot].
            src is PSUM f32; dst SBUF."""
            acc = stt[0:parts, slot:slot + 1] if stt is not None else None
            if residual is not None:
                bal.v += 1.042 * w + 200
                nc.vector.scalar_tensor_tensor(
                    dst, src, bias, residual, op0=ALU.add, op1=ALU.add,
                    accum_out=acc)
                return
            e = bal.pick(0.833 * w + 250, 1.042 * w + 200)
            if e == "a":
                nc.scalar.activation(dst, src, AF.Identity, bias=bias,
                                     accum_out=acc)
            elif acc is not None:
                nc.vector.tensor_scalar(dst, src, bias, 0.0, op0=ALU.add,
                                        op1=ALU.add, accum_out=acc)
            else:
                nc.vector.tensor_scalar(dst, src, bias, None, op0=ALU.add)

        def relu_evac(dst, src, bias, w=432, parts=128):
            e = bal.pick(0.833 * w + 250, 2 * (0.52 * w + 130))
            if e == "a":
                nc.scalar.activation(dst, src, AF.Relu, bias=bias)
            else:
                nc.vector.tensor_scalar(dst, src, bias, None, op0=ALU.add)
                nc.vector.tensor_scalar_max(dst, dst, 0.0)

        def relu_scale_chunk(dst, src, a_ap, b_ap, w):
            e = bal.pick(0.833 * w + 250, 2 * (0.45 * w + 130))
            if e == "a":
                nc.scalar.activation(dst, src, AF.Relu, bias=b_ap,
                                     scale=a_ap)
            else:
                nc.vector.tensor_scalar(dst, src, a_ap, b_ap, op0=ALU.mult,
                                        op1=ALU.add)
                nc.vector.tensor_scalar_max(dst, dst, 0.0)

        def relu_apply(row_ap_fn, a_ap, b_ap, parts=128, src_fn=None):
            """y = relu(a*src + b) over a full row, split main/tail.
            In-place when src_fn is None."""
            if src_fn is None:
                src_fn = row_ap_fn
            for (f0, fw) in ((0, ROWM), (ROWM, N - ROWM)):
                ap = row_ap_fn(f0, fw)
                sp_ = src_fn(f0, fw)
                e = bal.pick(0.833 * fw + 250, 2 * (0.3 * fw + 130))
                if e == "a":
                    nc.scalar.activation(ap, sp_, AF.Relu, bias=b_ap,
                                         scale=a_ap)
                else:
                    nc.vector.tensor_scalar(ap, sp_, a_ap, b_ap, op0=ALU.mult,
                                            op1=ALU.add)
                    nc.vector.tensor_scalar_max(ap, ap, 0.0)

        def square_stats(row_ap_fn, stt, parts=128):
            """Accumulate sum(x^2) of a row into stt slots 4,5 (pad excl)."""
            for j, (f0, fw) in enumerate(((0, ROWM), (ROWM, N - ROWM))):
                ap = row_ap_fn(f0, fw)
                scr = scp.tile([128, ROWM], F16, tag="scr", bufs=3,
                               name="scr")
                e = bal.pick(0.833 * fw + 250, 0.3 * fw + 130)
                acc = stt[0:parts, 4 + j:5 + j]
                if e == "a":
                    nc.scalar.activation(scr[0:parts, 0:fw], ap, AF.Square,
                                         accum_out=acc)
                else:
                    nc.vector.scalar_tensor_tensor(
                        scr[0:parts, 0:fw], ap, 1.0, ap, op0=ALU.mult,
                        op1=ALU.mult, accum_out=acc)

        def new_st(tag="st", n=1):
            return [sm.tile([128, 8], F32, tag=tag, bufs=10, name="st")
                    for _ in range(n)]

        def gn_chain(sts, gname, bname, parts=128, G=16, abtag="ab",
                     abbufs=4, gt0=0):
            """Batched GN chain over T=len(sts) channel tiles.
            sts[t] holds [sum0,sum1,sum2,sum3, sq0,sq1] per channel.
            Returns ab [parts, T, 2] with per-channel [a, beta]."""
            T = len(sts)
            stc = sm.tile([128, 8, 6], F32, tag="stc", bufs=4, name="stc")
            for t, stt in enumerate(sts):
                nc.vector.tensor_copy(stc[0:parts, t, :], stt[0:parts, 0:6])
            psg = ps.tile([16, 8, 6], F32, tag="ptr", bufs=2, name="psg")
            nc.tensor.matmul(psg[0:G, 0:T, :], g8n[0:parts, 0:G],
                             stc[0:parts, 0:T, :], start=True, stop=True)
            mr = sm.tile([16, 8, 2], F32, tag="mr", bufs=4, name="mr")
            nc.vector.tensor_reduce(mr[0:G, 0:T, 0:1], psg[0:G, 0:T, 0:4],
                                    mybir.AxisListType.X, ALU.add)
            e2 = sm.tile([16, 8], F32, tag="e2", bufs=4, name="e2")
            nc.vector.tensor_reduce(e2[0:G, 0:T].unsqueeze(-1),
                                    psg[0:G, 0:T, 4:6],
                                    mybir.AxisListType.X, ALU.add)
            sq = sm.tile([16, 8], F32, tag="sq", bufs=4, name="sq")
            nc.vector.tensor_tensor(sq[0:G, 0:T], mr[0:G, 0:T, 0],
                                    mr[0:G, 0:T, 0], op=ALU.mult)
            nc.vector.tensor_tensor(e2[0:G, 0:T], e2[0:G, 0:T], sq[0:G, 0:T],
                                    op=ALU.subtract)
            nc.scalar.activation(e2[0:G, 0:T], e2[0:G, 0:T], AF.Sqrt,
                                 bias=eps[0:G, :])
            nc.vector.reciprocal(mr[0:G, 0:T, 1], e2[0:G, 0:T])
            psb = ps.tile([128, 8, 2], F32, tag="ptr", bufs=2, name="psb")
            nc.tensor.matmul(psb[0:parts, 0:T, :], g8t[0:G, 0:parts],
                             mr[0:G, 0:T, :], start=True, stop=True)
            ab = sm.tile([128, 8, 2], F32, tag=abtag, bufs=abbufs, name="ab")
            gv = sm.tile([128, 8], F32, tag="gv", bufs=4, name="gv")
            for t in range(T):
                nc.vector.tensor_copy(gv[0:parts, t:t + 1],
                                      P(gname, gt0 + t, parts))
            nc.vector.tensor_tensor(ab[0:parts, 0:T, 0], psb[0:parts, 0:T, 1],
                                    gv[0:parts, 0:T], op=ALU.mult)
            t3 = sm.tile([128, 8], F32, tag="t3", bufs=4, name="t3")
            nc.vector.tensor_tensor(t3[0:parts, 0:T], psb[0:parts, 0:T, 0],
                                    ab[0:parts, 0:T, 0], op=ALU.mult)
            for t in range(T):
                nc.vector.tensor_copy(gv[0:parts, t:t + 1],
                                      P(bname, gt0 + t, parts))
            nc.vector.tensor_tensor(ab[0:parts, 0:T, 1], gv[0:parts, 0:T],
                                    t3[0:parts, 0:T], op=ALU.subtract)
            return ab

        # ---------- phases ----------
        def b0front(S, b):
            _mark("b0front")
            y1 = S["y1"]
            x = S["x"]
            y1st = new_st(n=2)
            xst = None
            for ci, (f0, fw) in enumerate(MCH):
                y1ps = [ps.tile([128, 432], F32, tag="pacc", bufs=2, name="y1ps")
                        for _ in range(2)]
                upss = {}

                def umm(kt):
                    upss[kt] = ps.tile([128, 432], F32, tag="ptr", bufs=2,
                                       name="ups")
                    nc.tensor.matmul(upss[kt][:, :fw],
                                     wsm[:, kt * 128:(kt + 1) * 128],
                                     verts[:, f0:f0 + fw],
                                     start=True, stop=True)

                umm(0)
                for kt in range(8):
                    if kt < 7:
                        umm(kt + 1)
                    xr = xrp.tile([128, 432], F16, tag="xr8", bufs=4,
                                  name="x0r")
                    relu_scale_chunk(xr[:, :fw], upss[kt][:, :fw],
                                     P(f"ab0a{b}", kt), P(f"ab0b{b}", kt), fw)
                    del upss[kt]
                    for mt in range(2):
                        nc.tensor.matmul(y1ps[mt][:, :fw],
                                         W(f"b0l1_{kt}")[:, mt * 128:
                                                         (mt + 1) * 128],
                                         xr[:, :fw],
                                         start=(kt == 0), stop=(kt == 7))
                for mt in range(4):
                    skps = ps.tile([128, 432], F32, tag="ptr", bufs=2, name="skps")
                    nc.tensor.matmul(skps[:, :fw],
                                     wsm[:, 1024 + mt * 128:1024 +
                                         (mt + 1) * 128],
                                     verts[:, f0:f0 + fw], start=True,
                                     stop=True)
                    rw = fw if f0 + fw <= N else (N - f0)
                    evac(x[mt][:, f0:f0 + rw], skps[:, :rw],
                         P(f"skb{b}", mt), w=rw)
                for mt in range(2):
                    rw = fw if f0 + fw <= N else (N - f0)
                    evac(y1[mt][:, f0:f0 + rw], y1ps[mt][:, :rw],
                         P("b0_lin1_b", mt), stt=y1st[mt], slot=ci, w=rw)
            for mt in range(2):
                square_stats(lambda f0, fwx, m=mt: y1[m][:, f0:f0 + fwx],
                             y1st[mt])
            S["y1st"] = y1st

        def lin1(S, i):
            _mark("lin1")
            y1 = S["y1"]
            x = S["x"]
            abx = S["xab"]
            y1st = new_st(n=2)
            xrf = [xrp.tile([128, NP], F16, tag="xrf", bufs=5, name="xrf")
                   for _ in range(4)]
            for ct in range(4):
                relu_apply(lambda f0, fwx, c=ct: xrf[c][:, f0:f0 + fwx],
                           abx[:, ct, 0:1], abx[:, ct, 1:2], src_fn=lambda
                           f0, fwx, c=ct: x[c][:, f0:f0 + fwx])
                nc.vector.memset(xrf[ct][:, N:NP], 0.0)
            for ci, (f0, fw) in enumerate(MCH):
                y1ps = [ps.tile([128, 432], F32, tag="pacc", bufs=2, name="y1psb")
                        for _ in range(2)]
                for ct in range(4):
                    for mt in range(2):
                        nc.tensor.matmul(
                            y1ps[mt][:, :fw],
                            W(f"bl1_{i}_{ct}")[:, mt * 128:(mt + 1) * 128],
                            xrf[ct][:, f0:f0 + fw], start=(ct == 0),
                            stop=(ct == 3))
                for mt in range(2):
                    rw = fw if f0 + fw <= N else (N - f0)
                    evac(y1[mt][:, f0:f0 + rw], y1ps[mt][:, :rw],
                         P(f"lin1_b{i}", mt), stt=y1st[mt], slot=ci, w=rw)
            for mt in range(2):
                square_stats(lambda f0, fwx, m=mt: y1[m][:, f0:f0 + fwx],
                             y1st[mt])
            S["y1st"] = y1st

        def sup_phase(S, pn_n1g, pn_n1b, cwn):
            _mark("sup")
            y1 = S["y1"]
            sup = S["sup"]
            supr = S.get("supr")
            ab = gn_chain(S["y1st"], pn_n1g, pn_n1b, abtag="aby", abbufs=4)
            for mt in range(2):
                relu_apply(lambda f0, fwx, m=mt: y1[m][:, f0:f0 + fwx],
                           ab[:, mt, 0:1], ab[:, mt, 1:2])
            for g0 in range(0, NT, 2):
                g1, rows, ng = g0 + 2, 128, 2
                sps = ps.tile([128, 2, 512], F32, tag="psup", bufs=2,
                              name="sps")
                for ct in range(2):
                    for nt in range(g0, g1):
                        ms = nt * 128
                        mw = min(ms + 128, NP) - ms
                        nc.tensor.matmul(sps[0:mw, nt - g0, 0:256],
                                         y1[ct][:, ms:ms + mw],
                                         W(f"{cwn}_{ct}"),
                                         start=(ct == 0), stop=(ct == 1))
                # last tile pair: nt=13 only has 60 valid rows; evac per nt
                parts_list = ([(0, 2, 128)] if g0 < 12 else
                              [(0, 1, 128), (1, 2, 60)])
                for (j0, j1, rr) in parts_list:
                    nels = (j1 - j0) * 256
                    sview = sps[0:rr, j0:j1, 0:256]
                    if agg_fp8:
                        dst = sup[0:rr, g0 + j0:g0 + j1, :]
                        e = bal.pick(0.833 * nels + 250, 1.042 * nels + 200)
                        if e == "a":
                            nc.scalar.activation(dst, sview, AF.Copy,
                                                 scale=SUPS)
                        else:
                            nc.vector.tensor_scalar(dst, sview, SUPS, None,
                                                    op0=ALU.mult)
                        # error-feedback residual r = f8(s*sup - q1)
                        bal.v += 1.042 * nels + 200
                        nc.vector.scalar_tensor_tensor(
                            supr[0:rr, g0 + j0:g0 + j1, :], sview, SUPS,
                            dst, op0=ALU.mult, op1=ALU.subtract)
                    else:
                        dst = sup[0:rr, g0 + j0:g0 + j1, :]
                        e = bal.pick(0.833 * nels + 250, 1.042 * nels + 200)
                        if e == "a":
                            nc.scalar.activation(dst, sview, AF.Copy)
                        else:
                            nc.vector.tensor_copy(dst, sview)

        def agg_phase(S, pn_cb, pn_n2g, pn_n2b):
            _mark("agg")
            y2 = S["y2"]
            sup = S["sup"]
            supr = S.get("supr")
            y2st = new_st(n=2)
            scale = (1.0 / SUPS) if agg_fp8 else 1.0
            for dt in range(2):
                for ci, (f0, fw) in enumerate(MCH):
                    aps = ps.tile([128, 432], F32, tag="pacc", bufs=2, name="aps")
                    if agg_fp8:
                        for gi, sp in enumerate((sup, supr)):
                            for k2 in range(NT2):
                                nc.tensor.matmul(
                                    aps[:, :fw],
                                    sp[:, 2 * k2:2 * k2 + 2,
                                       dt * 128:(dt + 1) * 128],
                                    at8[:, k2, :, f0:f0 + fw],
                                    start=(gi == 0 and k2 == 0),
                                    stop=(gi == 1 and k2 == NT2 - 1),
                                    perf_mode=mybir.MatmulPerfMode.DoubleRow)
                    else:
                        for kt in range(NT):
                            kn = min(128, N - kt * 128)
                            nc.tensor.matmul(
                                aps[:, :fw],
                                sup[0:kn, kt, dt * 128:(dt + 1) * 128],
                                asb[0:kn, kt, f0:f0 + fw],
                                start=(kt == 0), stop=(kt == NT - 1))
                    rw = fw if f0 + fw <= N else (N - f0)
                    # evac with bias and 1/SUPS scale
                    acc = y2st[dt][0:128, ci:ci + 1]
                    e = bal.pick(0.833 * rw + 250, 1.042 * rw + 200)
                    if e == "a" or not agg_fp8:
                        nc.scalar.activation(y2[dt][:, f0:f0 + rw],
                                             aps[:, :rw], AF.Identity,
                                             bias=P(pn_cb, dt), scale=scale,
                                             accum_out=acc)
                    else:
                        nc.vector.tensor_scalar(y2[dt][:, f0:f0 + rw],
                                                aps[:, :rw], scale,
                                                P(pn_cb, dt), op0=ALU.mult,
                                                op1=ALU.add, accum_out=acc)
                square_stats(lambda f0, fwx, m=dt: y2[m][:, f0:f0 + fwx],
                             y2st[dt])
            S["_y2st"] = y2st
            S["_n2"] = (pn_n2g, pn_n2b)

        def gn3_phase(S):
            _mark("gn3")
            y2 = S["y2"]
            y2st = S["_y2st"]
            pn_n2g, pn_n2b = S["_n2"]
            abs_ = [gn_chain([y2st[dt]], pn_n2g, pn_n2b, abtag="aby",
                             abbufs=4, gt0=dt) for dt in range(2)]
            for dt in range(2):
                relu_apply(lambda f0, fwx, m=dt: y2[m][:, f0:f0 + fwx],
                           abs_[dt][:, 0, 0:1], abs_[dt][:, 0, 1:2])

        def lin2_phase(S, l2n, pn_l2b, pn_gnext, collect):
            _mark("lin2")
            x = S["x"]
            y2 = S["y2"]
            xst = new_st(tag="stx", n=4) if collect else None
            for ci, (f0, fw) in enumerate(MCH):
                rw = fw if f0 + fw <= N else (N - f0)
                for mt in range(4):
                    lps = ps.tile([128, 432], F32, tag="pacc", bufs=2, name="lps")
                    for ct in range(2):
                        nc.tensor.matmul(
                            lps[:, :fw],
                            W(f"{l2n}_{ct}")[:, mt * 128:(mt + 1) * 128],
                            y2[ct][:, f0:f0 + fw],
                            start=(ct == 0), stop=(ct == 1))
                    evac(x[mt][:, f0:f0 + rw], lps[:, :rw], P(pn_l2b, mt),
                         stt=xst[mt] if collect else None, slot=ci,
                         residual=x[mt][:, f0:f0 + rw], w=rw)
            if collect:
                for mt in range(4):
                    square_stats(lambda f0, fwx, m=mt: x[m][:, f0:f0 + fwx],
                                 xst[mt])
                S["_xst"] = xst
                S["_gnext"] = pn_gnext
            else:
                S["xab"] = None
                S["_xst"] = None

        def gnx_phase(S):
            if S.get("_xst") is None:
                return
            _mark("gnx")
            g, bn = S["_gnext"]
            S["xab"] = gn_chain(S["_xst"], g, bn, abtag="abx", abbufs=2)
            S["_xst"] = None

        def head_a(S):
            _mark("head")
            x = S["x"]
            yh1 = S["yh1"]
            for (f0, fw) in MCH:
                rw = fw if f0 + fw <= N else (N - f0)
                hps = ps.tile([64, 432], F32, tag="pacc", bufs=2, name="hps")
                for kt in range(4):
                    nc.tensor.matmul(hps[:, :fw], W(f"h1_{kt}"),
                                     x[kt][:, f0:f0 + fw],
                                     start=(kt == 0), stop=(kt == 3))
                relu_evac(yh1[:, f0:f0 + rw], hps[0:64, :rw],
                          P("h1_b", 0, 64), w=rw, parts=64)

        def head_b(S):
            _mark("head")
            yh1, yh2 = S["yh1"], S["yh2"]
            hst = new_st(n=1)
            for ci, (f0, fw) in enumerate(MCH):
                rw = fw if f0 + fw <= N else (N - f0)
                hps2 = ps.tile([32, 432], F32, tag="pacc", bufs=2, name="hps2")
                nc.tensor.matmul(hps2[:, :fw], W("h2", 64), yh1[:, f0:f0 + fw],
                                 start=True, stop=True)
                evac(yh2[:, f0:f0 + rw], hps2[0:32, :rw], P("h2_b", 0, 32),
                     stt=hst[0], slot=ci, w=rw, parts=32)
            square_stats(lambda f0, fwx: yh2[:, f0:f0 + fwx], hst[0],
                         parts=32)
            S["_hst"] = hst

        def head_c(S):
            _mark("head")
            yh2 = S["yh2"]
            abh = gn_chain(S["_hst"], "hn_g", "hn_b", parts=32, G=4,
                           abtag="abh")
            relu_apply(lambda f0, fwx: yh2[:, f0:f0 + fwx],
                       abh[0:32, 0, 0:1], abh[0:32, 0, 1:2], parts=32)

        def head_d(S, b):
            _mark("head")
            yh2, osb = S["yh2"], S["osb"]
            for (f0, fw) in MCH:
                rw = fw if f0 + fw <= N else (N - f0)
                hps3 = ps.tile([4, 432], F32, tag="pacc", bufs=2, name="hps3")
                nc.tensor.matmul(hps3[:, :fw], W("h3", 32), yh2[:, f0:f0 + fw],
                                 start=True, stop=True)
                evac(osb[0:3, f0:f0 + rw], hps3[0:3, :rw], P("h3_b", 0, 3),
                     w=rw, parts=3)
            nc.sync.dma_start(out_d.ap()[b], osb[0:3, 0:N])

        rep = tc.For_i(0, nreps, 1) if nreps > 1 else contextlib.nullcontext()
        with rep:
            for b in range(BLOC):
                b0front(bt[b], b)
            for b in range(BLOC):
                sup_phase(bt[b], "b0_n1_g", "b0_n1_b", "b0cw")
            for b in range(BLOC):
                agg_phase(bt[b], "b0_conv_b", "b0_n2_g", "b0_n2_b")
            for b in range(BLOC):
                gn3_phase(bt[b])
            for b in range(BLOC):
                lin2_phase(bt[b], "b0l2", "b0_lin2_b",
                           ("blk0_pre_g", "blk0_pre_b"), True)
            for b in range(BLOC):
                gnx_phase(bt[b])
            for i in range(L):
                for b in range(BLOC):
                    lin1(bt[b], i)
                for b in range(BLOC):
                    sup_phase(bt[b], f"n1_g{i}", f"n1_b{i}", f"bcw_{i}")
                for b in range(BLOC):
                    agg_phase(bt[b], f"conv_b{i}", f"n2_g{i}", f"n2_b{i}")
                for b in range(BLOC):
                    gn3_phase(bt[b])
                for b in range(BLOC):
                    collect = i < L - 1
                    nxt = (f"pre_g{i + 1}", f"pre_b{i + 1}") if collect \
                        else (None, None)
                    lin2_phase(bt[b], f"bl2_{i}", f"lin2_b{i}", nxt, collect)
                for b in range(BLOC):
                    gnx_phase(bt[b])
            for b in range(BLOC):
                head_a(bt[b])
            for b in range(BLOC):
                head_b(bt[b])
            for b in range(BLOC):
                head_c(bt[b])
            for b in range(BLOC):
                head_d(bt[b], b)

    nc.compile()
    return nc


def _host_prep(inputs, agg_fp8=AGG_FP8):
    f32 = np.float32
    shared = {}

    verts_in = np.asarray(inputs["ref_vertices"], f32)
    verts = np.zeros((4, NP), f32)
    verts[0:3, 0:N] = verts_in
    shared["verts"] = verts

    lin0_W = np.asarray(inputs["lin0_W"], f32)
    w0v = lin0_W[:, 0:3]                       # (1024, 3)
    skW = np.asarray(inputs["b0_skip_W"], f32)  # (512, 1024)
    wsm = np.zeros((4, 1536), f32)
    wsm[0:3, 0:1024] = w0v.T
    wsm[0:3, 1024:1536] = (skW @ w0v).T        # skv
    shared["wsm"] = wsm

    # adjacency
    src = np.asarray(inputs["adj_src"]).astype(np.int64)
    dst = np.asarray(inputs["adj_dst"]).astype(np.int64)
    w = np.asarray(inputs["adj_w"], f32)
    at = np.zeros((NT * 128, NP), f32)
    np.add.at(at, (src, dst), w)
    if agg_fp8:
        a8 = at.reshape(NT2, 2, 128, NP).transpose(2, 0, 1, 3)
        shared["at8"] = np.ascontiguousarray(a8).astype(
            mybir.dt.np(FP8))
    else:
        atp = at.reshape(NT, 128, NP).transpose(1, 0, 2)
        shared["atp"] = np.ascontiguousarray(atp).astype(np.float16)

    # weights concat (bf16)
    wcat = np.zeros((128, WCOLS), f32)

    def put(name, arr):
        p0, wd = _WC[name]
        pr, wc = arr.shape
        assert wc == wd, (name, arr.shape)
        wcat[0:pr, p0:p0 + wd] = arr

    b0l1 = np.asarray(inputs["b0_lin1_W"], f32).T      # (1024, 256)
    for kt in range(8):
        put(f"b0l1_{kt}", b0l1[kt * 128:(kt + 1) * 128])
    b0cw = np.asarray(inputs["b0_conv_W"], f32)        # (256, 256)
    put("b0cw_0", b0cw[0:128]); put("b0cw_1", b0cw[128:256])
    b0l2 = np.asarray(inputs["b0_lin2_W"], f32).T      # (256, 512)
    put("b0l2_0", b0l2[0:128]); put("b0l2_1", b0l2[128:256])
    bl1 = np.asarray(inputs["blk_lin1_W"], f32)        # (L, 256, 512)
    bcw = np.asarray(inputs["blk_conv_W"], f32)        # (L, 256, 256)
    bl2 = np.asarray(inputs["blk_lin2_W"], f32)        # (L, 512, 256)
    for i in range(L):
        t1 = bl1[i].T                                  # (512, 256)
        for ct in range(4):
            put(f"bl1_{i}_{ct}", t1[ct * 128:(ct + 1) * 128])
        put(f"bcw_{i}_0", bcw[i][0:128]); put(f"bcw_{i}_1", bcw[i][128:256])
        t2 = bl2[i].T                                  # (256, 512)
        put(f"bl2_{i}_0", t2[0:128]); put(f"bl2_{i}_1", t2[128:256])
    h1 = np.asarray(inputs["h1_W"], f32).T             # (512, 64)
    for kt in range(4):
        put(f"h1_{kt}", h1[kt * 128:(kt + 1) * 128])
    put("h2", np.asarray(inputs["h2_W"], f32).T)       # (64, 32)
    h3 = np.zeros((32, 4), f32)
    h3[:, 0:3] = np.asarray(inputs["h3_W"], f32).T
    put("h3", h3)
    shared["wcat"] = wcat.astype(np.float16)

    ind = np.zeros((128, 16), f32)
    for c in range(128):
        ind[c, c // 8] = 1.0
    shared["g8n"] = ind / (8.0 * N)
    shared["g8t"] = np.ascontiguousarray(ind.T)

    # host-side lin0 / b0-GN1 analytics
    img = np.asarray(inputs["image_resnet"], f32)      # (B, 2048)
    vb_all = lin0_W[:, 3:] @ img.T + \
        np.asarray(inputs["lin0_b"], f32)[:, None]     # (1024, B)
    U = lin0_W[:, 0:3] @ verts_in                      # (1024, N)
    mU = U.mean(1)
    e2U = (U * U).mean(1)
    pre_g = np.asarray(inputs["b0_pre_g"], f32)
    pre_b = np.asarray(inputs["b0_pre_b"], f32)
    skb = np.asarray(inputs["b0_skip_b"], f32)

    vals = {"b0_lin1_b": inputs["b0_lin1_b"],
            "b0_n1_g": inputs["b0_n1_g"], "b0_n1_b": inputs["b0_n1_b"],
            "b0_conv_b": inputs["b0_conv_b"],
            "b0_n2_g": inputs["b0_n2_g"], "b0_n2_b": inputs["b0_n2_b"],
            "b0_lin2_b": inputs["b0_lin2_b"],
            "blk0_pre_g": np.asarray(inputs["blk_pre_g"])[0],
            "blk0_pre_b": np.asarray(inputs["blk_pre_b"])[0],
            "h1_b": inputs["h1_b"], "h2_b": inputs["h2_b"],
            "hn_g": inputs["hn_g"], "hn_b": inputs["hn_b"],
            "h3_b": inputs["h3_b"]}
    for i in range(L):
        vals[f"lin1_b{i}"] = np.asarray(inputs["blk_lin1_b"])[i]
        vals[f"n1_g{i}"] = np.asarray(inputs["blk_n1_g"])[i]
        vals[f"n1_b{i}"] = np.asarray(inputs["blk_n1_b"])[i]
        vals[f"conv_b{i}"] = np.asarray(inputs["blk_conv_b"])[i]
        vals[f"n2_g{i}"] = np.asarray(inputs["blk_n2_g"])[i]
        vals[f"n2_b{i}"] = np.asarray(inputs["blk_n2_b"])[i]
        vals[f"lin2_b{i}"] = np.asarray(inputs["blk_lin2_b"])[i]
        if i < L - 1:
            vals[f"pre_g{i + 1}"] = np.asarray(inputs["blk_pre_g"])[i + 1]
            vals[f"pre_b{i + 1}"] = np.asarray(inputs["blk_pre_b"])[i + 1]

    in_maps = []
    for core in range(NCORES):
        vals_c = dict(vals)
        for bl in range(BLOC):
            bglob = core * BLOC + bl
            vb = vb_all[:, bglob]                      # (1024,)
            m_c = mU + vb
            e2_c = e2U + 2 * mU * vb + vb * vb
            m_g = m_c.reshape(128, 8).mean(1)
            e2_g = e2_c.reshape(128, 8).mean(1)
            var_g = e2_g - m_g * m_g
            rs = 1.0 / np.sqrt(var_g + 1e-5)
            a_c = pre_g * np.repeat(rs, 8)
            b_c = pre_b - np.repeat(m_g * rs, 8) * pre_g
            vals_c[f"ab0a{bl}"] = a_c
            vals_c[f"ab0b{bl}"] = b_c + a_c * vb
            vals_c[f"skb{bl}"] = skW @ vb + skb
        prm = np.zeros((128, NSLOT), f32)
        for (name, t), pos in PIDX.items():
            vec = np.asarray(vals_c[name], f32).ravel()
            seg = vec[t * 128:(t + 1) * 128]
            prm[0:len(seg), pos] = seg
        m = dict(shared)
        m["prm"] = prm
        in_maps.append(m)
    return in_maps


_NC_CACHE = {}


def _get_nc(nreps=1, **kw):
    key = (nreps, tuple(sorted(kw.items())))
    if key not in _NC_CACHE:
        _NC_CACHE[key] = build(nreps, **kw)
    return _NC_CACHE[key]


def run_on_hw(inputs, nreps=1, **kw):
    nc = _get_nc(nreps, **kw)
    in_maps = _host_prep(inputs, agg_fp8=kw.get("agg_fp8", AGG_FP8))
    res = run_bass_kernel_spmd(nc, in_maps, core_ids=list(range(NCORES)),
                               trace=False)
    return np.concatenate([res.results[c]["out"] for c in range(NCORES)],
                          axis=0)


def kernel(**inputs) -> np.ndarray:
    return run_on_hw(inputs, nreps=1)
